# revision 19
# baseline (speedup 1.0000x reference)
"""AttnBlock (GroupNorm + single-head spatial attention + residual) on 8 trn2
NeuronCores, data-parallel over batch (1 image per core).

Per-core plan (image = x[b] viewed as [C=768, N=4096] fp32):
  A) QKV projections start as soon as x chunks arrive: GroupNorm is folded
     algebraically -- qkv = sum_g rstd_g * (wqkv*gn_w)[g-chunks].T @ x[g]
     + (qkvb + wqkv.T@gn_b - sum_g rstd_g*mean_g*rowsum_g) -- so the matmuls
     are stats-independent and only the cheap per-nb combine waits on the
     bn_stats reduction.
  B) q,k replicated at partition offsets {0,32,64,96} for 4-way row-packed
     (tile_position) QK matmuls; v transposed on PE into vT_aug32[j, 32]
     (cols 0..15 = v.T, col 16 = ones for softmax denominators, rest zero).
  C) Attention per 512-wide i-block: scores computed TRANSPOSED
     sT[j, i] = k.T q (exp needs no transpose; max-subtraction skipped --
     scores are provably small here), exp on ACT straight out of PSUM, AV
     matmuls 4-way column-packed: col-group r accumulates j-blocks =r (mod 4)
     into acc[32r:32r+32]. The 4 partial sums merge for free inside the 1x1
     projection (pwT stacked 4x with zero padding, K=128 contraction).
     Softmax denominators ride along as ones-column rows {16,48,80,112}.
"""

import numpy as np

_CACHE = {}

B, C, HW = 8, 768, 4096
RC = 16
NCH = 6  # C chunks of 128
NIB = 8  # i blocks of 512
NJB = 32  # j blocks of 128
EPS = 1e-6
SCALE = RC ** (-0.5)


def _apply_drain_patch():
    """This walrus build rejects ANY instruction carrying >1 sync-wait command
    (setupSyncWait: "Too many sync wait commands"). Two patches:
    1. _lower_ordered_insts: for every scheduled instruction with N>1 waits,
       keep one and move the rest onto nofuse NOPs inserted just before it on
       the same engine queue (sem-ge waits are absolute, so order-insensitive).
    2. _drain_and_barrier: same split for the kernel-tail drain, which
       aggregates the global clock."""
    import concourse.tile as tile_mod
    from concourse.vector_clock import ScopedClock

    if getattr(tile_mod.TileContext, "_drain_patched", False):
        return

    def _split_waits(self, insts, by_num):
        new_list = []
        for inst in insts:
            si = inst.sync_info
            waits = list(si.on_wait) if si and si.on_wait else []
            if len(waits) > 1:
                movable = [
                    w
                    for w in waits
                    if w.wait_reg is None and w.id in by_num
                ]
                kept = [w for w in waits if w not in movable]
                if not kept and movable:
                    kept = [movable.pop(0)]
                inst.sync_info.on_wait = kept
                for w in movable:
                    nop = self.nc.engines[inst.engine].nop(nofuse=True)
                    nop.wait_op(by_num[w.id], w.wait_value, "sem-ge")
                    new_list.append(nop.ins)
            new_list.append(inst)
        insts[:] = new_list

    orig_lower = tile_mod.TileContext._lower_ordered_insts

    def _lower_ordered_insts(self, ordered):
        cb = self.nc._state.pop_inst_callback()
        try:
            by_num = {h.num: h for h in self.sems.allocated().values()}
            for insts in ordered.values():
                _split_waits(self, insts, by_num)
        finally:
            self.nc._state.push_inst_callback(cb)
        return orig_lower(self, ordered)

    def _drain_and_barrier(self, tick_clock, wait_clock):
        nc = self.nc
        drain_inst = nc.sync.drain()
        wait_clock.add_sem_waits(
            drain_inst.ins, ScopedClock({None: tick_clock.global_clock})
        )
        waits = list(drain_inst.ins.sync_info.on_wait or [])
        if len(waits) > 1:
            drain_inst.ins.sync_info.on_wait = waits[:1]
            by_num = {h.num: h for h in self.sems.allocated().values()}
            for w in waits[1:]:
                extra = nc.sync.drain()
                extra.wait_op(by_num[w.id], w.wait_value, "sem-ge")
        nc.all_engine_barrier()
        assert self.sems is not None
        popped = nc._tile_sem_poison_stack.pop()
        assert popped is self._sem_poison
        nc.clear_and_free_semaphores(list(self.sems.allocated().values()))
        nc.all_engine_barrier()

    tile_mod.TileContext._lower_ordered_insts = _lower_ordered_insts
    tile_mod.TileContext._drain_and_barrier = _drain_and_barrier
    tile_mod.TileContext._drain_patched = True


def _build_nc(repeat=1):
    import concourse.bass as bass
    import concourse.mybir as mybir
    import concourse.tile as tile

    _apply_drain_patch()
    f32 = mybir.dt.float32
    AF = mybir.ActivationFunctionType
    ALU = mybir.AluOpType

    nc = bass.Bass()
    x_d = nc.dram_tensor("x", [C, HW], f32, kind="ExternalInput")
    wqkvT_d = nc.dram_tensor("wqkvT", [C, 48], f32, kind="ExternalInput")
    qkvb_d = nc.dram_tensor("qkvb", [48, 1], f32, kind="ExternalInput")
    gnw_d = nc.dram_tensor("gnw", [C], f32, kind="ExternalInput")
    gnb_d = nc.dram_tensor("gnb", [C], f32, kind="ExternalInput")
    pwT_d = nc.dram_tensor("pwT", [RC, C], f32, kind="ExternalInput")
    pb_d = nc.dram_tensor("pb", [C], f32, kind="ExternalInput")
    ident_d = nc.dram_tensor("ident", [RC, RC], f32, kind="ExternalInput")
    out_d = nc.dram_tensor("out", [C, HW], f32, kind="ExternalOutput")

    with tile.TileContext(nc) as tc:
      for _rep in range(repeat):
        with (
            tc.tile_pool(name="xpool", bufs=NCH) as xpool,
            tc.tile_pool(name="wts", bufs=1) as wts,
            tc.tile_pool(name="attn", bufs=1) as attn_pool,
            tc.tile_pool(name="ptiles", bufs=3) as ptiles,
            tc.tile_pool(name="norm", bufs=2) as norm_pool,
            tc.tile_pool(name="res", bufs=3) as res_pool,
        ):
            # ---------------- load x + weights ----------------
            x_sb = []
            for t in range(NCH):
                xt = xpool.tile([128, HW], f32, tag="x")
                nc.sync.dma_start(out=xt, in_=x_d[t * 128 : (t + 1) * 128, :])
                x_sb.append(xt)

            wq_sb = wts.tile([128, NCH, 48], f32)
            nc.sync.dma_start(
                out=wq_sb,
                in_=bass.AP(wqkvT_d, 0, [[48, 128], [48 * 128, NCH], [1, 48]]),
            )
            qkvb_sb = wts.tile([48, 1], f32)
            nc.sync.dma_start(out=qkvb_sb, in_=qkvb_d[:, :])
            gnw_sb = wts.tile([128, NCH], f32)
            nc.sync.dma_start(
                out=gnw_sb, in_=bass.AP(gnw_d, 0, [[1, 128], [128, NCH]])
            )
            gnb_sb = wts.tile([128, NCH], f32)
            nc.sync.dma_start(
                out=gnb_sb, in_=bass.AP(gnb_d, 0, [[1, 128], [128, NCH]])
            )
            # pwT stacked 4x at partition bands {0,32,64,96}, zeros elsewhere
            pwT4 = wts.tile([128, NCH, 128], f32)
            nc.vector.memset(pwT4, 0.0)
            for r in range(4):
                nc.sync.dma_start(
                    out=pwT4[32 * r : 32 * r + RC, :, :],
                    in_=bass.AP(pwT_d, 0, [[C, RC], [128, NCH], [1, 128]]),
                )
            pb_sb = wts.tile([128, NCH], f32)
            nc.sync.dma_start(
                out=pb_sb, in_=bass.AP(pb_d, 0, [[1, 128], [128, NCH]])
            )
            ident_sb = wts.tile([RC, RC], f32)
            nc.sync.dma_start(out=ident_sb, in_=ident_d[:, :])
            ones_col = wts.tile([128, 1], f32)
            nc.vector.memset(ones_col, 1.0)
            ones128 = wts.tile([1, 128], f32)
            nc.vector.memset(ones128, 1.0)
            ones4 = wts.tile([4, 1], f32)
            nc.vector.memset(ones4, 1.0)

            # ------- stats-independent PE work (overlaps the x DMA) -------
            bias1_sb = wts.tile([48, 1], f32)
            S_sb = wts.tile([48, 2], f32)
            qkv_sb = None
            with (
                tc.tile_pool(name="bps", bufs=1, space="PSUM") as bps,
                tc.tile_pool(name="qkvps", bufs=2, space="PSUM") as qkvps,
                tc.tile_pool(name="bcps", bufs=1, space="PSUM") as bcps,
                tc.tile_pool(name="qkvsb", bufs=1) as qkvsb_pool,
            ):
                bs_ps = bps.tile([48, 3], f32)
                # bias1 = wqkv.T @ gn_b with UNFOLDED weights
                for t in range(NCH):
                    nc.tensor.matmul(
                        out=bs_ps[:, 0:1],
                        lhsT=wq_sb[:, t, :],
                        rhs=gnb_sb[:, t : t + 1],
                        start=(t == 0),
                        stop=(t == NCH - 1),
                    )
                nc.vector.tensor_copy(out=bias1_sb, in_=bs_ps[:, 0:1])
                # fold gn_w into the weights in place
                for t in range(NCH):
                    nc.vector.tensor_scalar_mul(
                        out=wq_sb[:, t, :],
                        in0=wq_sb[:, t, :],
                        scalar1=gnw_sb[:, t : t + 1],
                    )
                # per-group row sums of the folded weights
                for g in range(2):
                    for i, t in enumerate(range(3 * g, 3 * g + 3)):
                        nc.tensor.matmul(
                            out=bs_ps[:, 1 + g : 2 + g],
                            lhsT=wq_sb[:, t, :],
                            rhs=ones_col,
                            start=(i == 0),
                            stop=(i == 2),
                        )
                nc.vector.tensor_copy(out=S_sb, in_=bs_ps[:, 1:3])
                # group-split QKV matmuls (no stats dependency)
                qkv_sb = qkvsb_pool.tile([48, HW], f32)
                q_ps = []
                for nb in range(NIB):
                    p0 = qkvps.tile([48, 512], f32, tag="q0", name="p0")
                    p1 = qkvps.tile([48, 512], f32, tag="q1", name="p1")
                    for i, t in enumerate(range(3)):
                        nc.tensor.matmul(
                            out=p0,
                            lhsT=wq_sb[:, t, :],
                            rhs=x_sb[t][:, nb * 512 : (nb + 1) * 512],
                            start=(i == 0),
                            stop=(i == 2),
                        )
                    for i, t in enumerate(range(3, 6)):
                        nc.tensor.matmul(
                            out=p1,
                            lhsT=wq_sb[:, t, :],
                            rhs=x_sb[t][:, nb * 512 : (nb + 1) * 512],
                            start=(i == 0),
                            stop=(i == 2),
                        )
                    q_ps.append((p0, p1))

                # ---------------- GroupNorm stats ----------------
                with tc.tile_pool(name="stats", bufs=4) as spool:
                    mv_sb = wts.tile([128, NCH, 2], f32)
                    for t in range(NCH):
                        st = spool.tile([128, 8, 6], f32, tag="st")
                        for s in range(8):
                            nc.vector.bn_stats(
                                out=st[:, s, :],
                                in_=x_sb[t][:, s * 512 : (s + 1) * 512],
                            )
                        nc.vector.bn_aggr(out=mv_sb[:, t, :], in_=st)

                    # gather all (mean, var) pairs onto one partition
                    g_sb = wts.tile([1, 128 * NCH * 2], f32)
                    gv = g_sb.rearrange(
                        "a (p t s) -> a p t s", p=128, t=NCH, s=2
                    )
                    nc.gpsimd.dma_start(out=gv, in_=mv_sb[:, :, :])

                    mg_sb = wts.tile([1, 2], f32)  # group means
                    rstd_sb = wts.tile([1, 2], f32)  # group rstds
                    eps_sb = wts.tile([1, 1], f32)
                    nc.vector.memset(eps_sb, EPS)
                    for g in range(2):
                        means = gv[:, :, 3 * g : 3 * g + 3, 0:1]
                        varis = gv[:, :, 3 * g : 3 * g + 3, 1:2]
                        tmp = spool.tile([1, 128, 3, 1], f32, tag="tmp")
                        nc.vector.tensor_mul(out=tmp, in0=means, in1=means)
                        nc.vector.tensor_add(out=tmp, in0=tmp, in1=varis)
                        ssum = spool.tile([1, 1], f32, tag="ssum")
                        msum = spool.tile([1, 1], f32, tag="msum")
                        nc.vector.reduce_sum(
                            out=ssum, in_=tmp, axis=mybir.AxisListType.XYZ
                        )
                        nc.vector.reduce_sum(
                            out=msum, in_=means, axis=mybir.AxisListType.XYZ
                        )
                        nc.vector.tensor_scalar_mul(
                            out=mg_sb[:, g : g + 1], in0=msum, scalar1=1.0 / 384.0
                        )
                        e2 = spool.tile([1, 1], f32, tag="e2")
                        nc.vector.tensor_scalar_mul(
                            out=e2, in0=ssum, scalar1=1.0 / 384.0
                        )
                        m2 = spool.tile([1, 1], f32, tag="m2")
                        nc.vector.tensor_mul(
                            out=m2,
                            in0=mg_sb[:, g : g + 1],
                            in1=mg_sb[:, g : g + 1],
                        )
                        nc.vector.tensor_sub(out=e2, in0=e2, in1=m2)
                        nc.scalar.activation(
                            out=e2, in_=e2, func=AF.Sqrt, bias=eps_sb[:, :]
                        )
                        nc.vector.reciprocal(
                            out=rstd_sb[:, g : g + 1], in_=e2
                        )

                    # PE-broadcast [r0, r1, m0, m1] to 48 partitions
                    st4 = wts.tile([1, 4], f32)
                    nc.vector.tensor_copy(out=st4[0:1, 0:2], in_=rstd_sb)
                    nc.vector.tensor_copy(out=st4[0:1, 2:4], in_=mg_sb)
                    rm48 = wts.tile([48, 4], f32)
                    bc_ps = bcps.tile([48, 4], f32)
                    nc.tensor.matmul(
                        out=bc_ps,
                        lhsT=ones128[:, 0:48],
                        rhs=st4,
                        start=True,
                        stop=True,
                    )
                    nc.vector.tensor_copy(out=rm48, in_=bc_ps)
                    # rmneg = -(rstd * mean) per group, broadcast on 48 rows
                    rmneg = wts.tile([48, 2], f32)
                    nc.vector.tensor_mul(
                        out=rmneg, in0=rm48[:, 0:2], in1=rm48[:, 2:4]
                    )
                    nc.vector.tensor_scalar_mul(
                        out=rmneg, in0=rmneg, scalar1=-1.0
                    )
                    # bias_tot = qkvb + bias1 - S0*r0*m0 - S1*r1*m1
                    bias_tot = wts.tile([48, 1], f32)
                    nc.vector.tensor_add(
                        out=bias_tot, in0=qkvb_sb, in1=bias1_sb
                    )
                    nc.vector.scalar_tensor_tensor(
                        out=bias_tot,
                        in0=S_sb[:, 0:1],
                        scalar=rmneg[:, 0:1],
                        in1=bias_tot,
                        op0=ALU.mult,
                        op1=ALU.add,
                    )
                    nc.vector.scalar_tensor_tensor(
                        out=bias_tot,
                        in0=S_sb[:, 1:2],
                        scalar=rmneg[:, 1:2],
                        in1=bias_tot,
                        op0=ALU.mult,
                        op1=ALU.add,
                    )

                # combine the group-split QKV partials
                for nb in range(NIB):
                    p0, p1 = q_ps[nb]
                    tq = norm_pool.tile([48, 512], f32, tag="tq")
                    nc.vector.tensor_scalar(
                        out=tq,
                        in0=p0,
                        scalar1=rm48[:, 0:1],
                        scalar2=bias_tot,
                        op0=ALU.mult,
                        op1=ALU.add,
                    )
                    nc.vector.scalar_tensor_tensor(
                        out=qkv_sb[:, nb * 512 : (nb + 1) * 512],
                        in0=p1,
                        scalar=rm48[:, 1:2],
                        in1=tq,
                        op0=ALU.mult,
                        op1=ALU.add,
                    )

                # replicate q, k to partition offsets 0/32/64/96
                qrep = attn_pool.tile([128, HW], f32)
                krep = attn_pool.tile([128, HW], f32)
                for r in range(4):
                    nc.sync.dma_start(
                        out=qrep[32 * r : 32 * r + RC, :], in_=qkv_sb[0:RC, :]
                    )
                    nc.sync.dma_start(
                        out=krep[32 * r : 32 * r + RC, :],
                        in_=qkv_sb[RC : 2 * RC, :],
                    )
                # v tiles to base-partition-0, transpose into vT_aug32
                vT_aug = attn_pool.tile([128, NJB, 32], f32)
                nc.vector.memset(vT_aug, 0.0)
                with tc.tile_pool(name="tps", bufs=2, space="PSUM") as tps:
                    for jb in range(NJB):
                        v_jb = qkvsb_pool.tile(
                            [RC, 128], f32, tag="vjb", bufs=4
                        )
                        nc.sync.dma_start(
                            out=v_jb,
                            in_=qkv_sb[2 * RC : 3 * RC, jb * 128 : (jb + 1) * 128],
                        )
                        tp = tps.tile([128, RC], f32, tag="tp")
                        nc.tensor.transpose(out=tp, in_=v_jb, identity=ident_sb)
                        nc.vector.tensor_copy(out=vT_aug[:, jb, 0:RC], in_=tp)
                nc.vector.memset(vT_aug[:, :, RC : RC + 1], 1.0)

            # ---------------- attention + proj ----------------
            with (
                tc.tile_pool(name="sps", bufs=2, space="PSUM") as sps,
                tc.tile_pool(name="accps", bufs=2, space="PSUM") as accps,
                tc.tile_pool(name="pjps", bufs=1, space="PSUM") as pjps,
                tc.tile_pool(name="nps", bufs=1, space="PSUM") as nps,
            ):
                def finalize(ib, acc):
                    ibs = slice(ib * 512, (ib + 1) * 512)
                    att4 = norm_pool.tile([128, 512], f32, tag="att4")
                    nc.vector.tensor_copy(out=att4, in_=acc)

                    # denominators: rows {16,48,80,112} of att4 -> total ->
                    # reciprocal on 128 lanes -> PE-broadcast to 128 rows
                    d4 = att4[RC : RC + 1, :]
                    den4 = norm_pool.tile([4, 512], f32, tag="den4")
                    nc.sync.dma_start(
                        out=den4,
                        in_=bass.AP(
                            d4.tensor, d4.offset, [[32 * 512, 4], [1, 512]]
                        ),
                    )
                    dt_ps = nps.tile([1, 512], f32, tag="nrm", name="dtps")
                    nc.tensor.matmul(
                        out=dt_ps, lhsT=ones4, rhs=den4, start=True, stop=True
                    )
                    dt_sb = norm_pool.tile([1, 512], f32, tag="dt")
                    nc.vector.tensor_copy(out=dt_sb, in_=dt_ps)
                    rec_in = norm_pool.tile([128, 4], f32, tag="rin")
                    nc.sync.dma_start(out=rec_in, in_=dt_sb)
                    rec_out = norm_pool.tile([128, 4], f32, tag="rout")
                    nc.vector.reciprocal(out=rec_out, in_=rec_in)
                    recrow = norm_pool.tile([1, 512], f32, tag="rrow")
                    nc.sync.dma_start(out=recrow, in_=rec_out)
                    nb_ps = nps.tile([128, 512], f32, tag="nrm", name="nbps")
                    nc.tensor.matmul(
                        out=nb_ps, lhsT=ones128, rhs=recrow, start=True, stop=True
                    )
                    nc.vector.tensor_mul(out=att4, in0=att4, in1=nb_ps)

                    # stacked projection (merges the 4 AV partials) + residual
                    for t in range(NCH):
                        pj = pjps.tile([128, 512], f32, tag="pj")
                        nc.tensor.matmul(
                            out=pj,
                            lhsT=pwT4[:, t, :],
                            rhs=att4,
                            start=True,
                            stop=True,
                        )
                        res = res_pool.tile([128, 512], f32, tag="res")
                        nc.vector.scalar_tensor_tensor(
                            out=res,
                            in0=pj,
                            scalar=pb_sb[:, t : t + 1],
                            in1=x_sb[t][:, ibs],
                            op0=ALU.add,
                            op1=ALU.add,
                        )
                        nc.sync.dma_start(
                            out=out_d[t * 128 : (t + 1) * 128, ibs], in_=res
                        )

                # software pipeline: emit attention(ib) before finalize(ib-1)
                # so the PE queue (in-order) never stalls on the normalize
                # chain at i-block boundaries
                pend = None
                for ib in range(NIB):
                    ibs = slice(ib * 512, (ib + 1) * 512)
                    acc = accps.tile([128, 512], f32, tag="acc")
                    for g in range(NIB):
                        s_h = [
                            sps.tile([128, 1024], f32, tag="s", name="s0"),
                            sps.tile([128, 1024], f32, tag="s", name="s1"),
                        ]
                        p_h = [
                            ptiles.tile([128, 1024], f32, tag="p", name="p0"),
                            ptiles.tile([128, 1024], f32, tag="p", name="p1"),
                        ]
                        for r in range(4):
                            jb = 4 * g + r
                            h, col = r // 2, (r % 2) * 512
                            nc.tensor.matmul(
                                out=s_h[h][:, col : col + 512],
                                lhsT=krep[
                                    32 * r : 32 * r + RC,
                                    jb * 128 : (jb + 1) * 128,
                                ],
                                rhs=qrep[32 * r : 32 * r + RC, ibs],
                                start=True,
                                stop=True,
                                tile_position=(32 * r, 0),
                            )
                        for h in range(2):
                            nc.scalar.activation(
                                out=p_h[h], in_=s_h[h], func=AF.Exp, scale=SCALE
                            )
                        # 4-way column-packed AV: col-group r accumulates
                        # j-blocks congruent to r (mod 4)
                        for r in range(4):
                            jb = 4 * g + r
                            h, col = r // 2, (r % 2) * 512
                            nc.tensor.matmul(
                                out=acc[32 * r : 32 * r + 32, :],
                                lhsT=vT_aug[:, jb, :],
                                rhs=p_h[h][:, col : col + 512],
                                start=(g == 0),
                                stop=(g == NIB - 1),
                                tile_position=(0, 32 * r),
                            )

                    if pend is not None:
                        finalize(*pend)
                    pend = (ib, acc)
                finalize(*pend)

    return nc


def kernel(x, gn_w, gn_b, qw, qb, kw, kb, vw, vb, pw, pb):
    from concourse.bass_utils import run_bass_kernel_spmd

    if "nc" not in _CACHE:
        _CACHE["nc"] = _build_nc()
    nc = _CACHE["nc"]

    xr = np.ascontiguousarray(x.reshape(B, C, HW).astype(np.float32))
    wqkvT = np.ascontiguousarray(
        np.concatenate([qw.T, kw.T, vw.T], axis=1).astype(np.float32)
    )
    qkvb = np.ascontiguousarray(
        np.concatenate([qb, kb, vb]).astype(np.float32).reshape(48, 1)
    )
    shared = {
        "wqkvT": wqkvT,
        "qkvb": qkvb,
        "gnw": np.ascontiguousarray(gn_w.astype(np.float32)),
        "gnb": np.ascontiguousarray(gn_b.astype(np.float32)),
        "pwT": np.ascontiguousarray(pw.T.astype(np.float32)),
        "pb": np.ascontiguousarray(pb.astype(np.float32)),
        "ident": np.eye(RC, dtype=np.float32),
    }
    in_maps = [dict(shared, x=xr[i]) for i in range(B)]
    res = run_bass_kernel_spmd(nc, in_maps, core_ids=list(range(B)))
    out = np.stack([res.results[i]["out"] for i in range(B)])
    return out.reshape(B, C, 64, 64).astype(np.float32)
